# revision 36
# baseline (speedup 1.0000x reference)
"""Multi-head self-attention (B=8, S=1024, E=768, H=12, D=64) on 8 NeuronCores.

Sharding: data-parallel over batch — one batch element per core, weights
replicated, no collectives.  v5; v1 measured ~233us HW exec, rel-RMS err
~1.6e-3 vs the fp32 reference.

Measured facts driving the design (NTFF profiles + probe kernels):
 - PE stream: fp16 moving operands cross the array at 1 cycle/row
   (~215ns for N=512 @2.4GHz); fp32 at 2 cycles/row.  All matmul
   operands are fp16.  Per-MM marginal cost ~257ns (N=512): ~40ns of
   exposed LDWEIGHTS per MM (walrus can't dedup or hide them fully;
   --enable-ldw-opt rejects pre-legalized LDWEIGHTS).
 - PE busy is the floor engine (~186us); ACT exp is second (12.6M exps
   at 1 col/cycle/partition @0.96GHz = ~107us).
 - XBAR DMA transposes (InstDmaTransposeAnt) were tried for x^T and
   REVERTED: the x-DMA -> DVE-cast -> xpose chain pays a DMA-completion
   semaphore lag (~2-6us) per stage, the scheduler reorders emission,
   and the limited DMA-semaphore pool serializes cross-queue — startup
   went 11us -> 29us.  PE transpose mode (~430ns/128x128 fp32 tile,
   LDW direct from the DMA'd x tile) wins the startup race.
 - Using ACT Ln anywhere forces walrus onto the combined
   natural_log_exp table which slows EVERY exp ~15% (1116 -> 1281ns) —
   softmax normalization stays on DVE (Newton + bit-trick seed).
 - DVE ops must be partition-aligned across operands and output
   (verifier NCC_IBIR297; DVE PSUM reads at a partition base different
   from the output's silently corrupt).  The normalization therefore
   swaps the SUMS rows (PSUM->SBUF DMAs on the idle gpsimd queue)
   right after attnV, making the whole Newton chain full-width and
   aligned: 5 DVE ops/unit instead of v1's 8 half-width ops.
 - nc.vector.reciprocal costs ~6.5 cycles/element — not used.
 - GPSIMD tensor_copy has ~2.7us overhead per call — weight f32->f16
   casts run on DVE; Wv stages through the gpsimd DMA queue (bufs=6,
   no ring WAR) so the sync queue carries only x + per-pair weights.

Per-core dataflow (layouts chosen so the only transpose is x -> xT):
  1. x s-tiles DMA in (one [128,768] DMA each, sync queue); xT = x^T
     via PE transpose (48 128x128 fp32 tiles), cast to fp16 on the
     PSUM->SBUF DVE evacuation.  Transposes interleave with V-proj
     per s-tile so PE never drains.
  2. V scattered into V_ext[s, ktile, head, 128] = [V_h+bv | ones]
     (even head) or [ones | V_h+bv] (odd head); ones-halves memset
     strided on gpsimd; adding bv here is exact (softmax rows sum to 1).
  3. QT/KT per head pair: lhsT=W tile, rhs=xT; bias via
     tensor_scalar_add on the PSUM evacuation (fp16 out).  The next
     pair's projection is emitted at the head of each (pair, q-half)
     unit so PE has independent work while ACT drains the previous
     unit's exps out of the S-ring.
  4. Per (pair, q-half): scores^T[k,q] = KT.T @ QT (row halves at
     partition base 0/64), exp on ACT with the 1/sqrt(D)=1/8 scale
     folded in (scores ~ N(0,1), no max guard needed).
  5. attnV: one M=128 matmul per (head, ktile) -> rows [attn^T|sums]
     (even) / [sums|attn^T] (odd).  Normalization: sums swapped into
     s_sw via 2 gpsimd DMAs, then seed/Newton/apply on DVE.
  6. out = concatT.T @ Wo + bo (bo broadcast via partition-step-0 DMA).
     Output s-tiles 0-3 only need q-half 0 of concatT: their matmuls
     are slotted between the last unit's scores and attnV, leaving
     s-tiles 4-7 plus one normalization chain for the tail.

Workaround baked in: this walrus build rejects instructions carrying
more than ~1-2 sync waits; _split_excess_waits and the patched
TileContext tail hoist surplus waits onto standalone EVSEM ops.
"""
import sys
sys.path.insert(0, "/opt/trn_rl_repo")
from contextlib import ExitStack

import numpy as np

import concourse.bass as bass
import concourse.tile as tile
from concourse import mybir
from concourse.bass_utils import run_bass_kernel_spmd
from concourse.masks import make_identity
from concourse.vector_clock import ScopedClock


def _split_drain_and_barrier(self, tick_clock, wait_clock):
    """TileContext tail with the final drain's waits split one-per-instruction."""
    drain_inst = self.nc.sync.drain()
    wait_clock.add_sem_waits(
        drain_inst.ins, ScopedClock({None: tick_clock.global_clock})
    )
    si = drain_inst.ins.sync_info
    waits = list(si.on_wait) if si is not None and si.on_wait else []
    if len(waits) > 1:
        si.on_wait = []
        by_num = {h.num: h for h in self.sems.allocated().values()}
        for w in waits:
            self.nc.sync.wait_ge(by_num[w.id], w.wait_value)
    self.nc.all_engine_barrier()
    popped = self.nc._tile_sem_poison_stack.pop()
    assert popped is self._sem_poison
    self.nc.clear_and_free_semaphores(list(self.sems.allocated().values()))
    self.nc.all_engine_barrier()


tile.TileContext._drain_and_barrier = _split_drain_and_barrier


def _split_excess_waits(nc):
    """Hoist excess per-instruction sync waits into standalone EVSEM waits
    (this walrus build rejects >1 wait per instruction, >2 per EVSEM)."""
    counter = 0
    for f in nc.m.functions:
        for bb in f.blocks:
            insts = bb.instructions
            out = []
            for inst in insts:
                si = inst.sync_info
                if "DmaTranspose" in type(inst).__name__:
                    cap = 0
                elif isinstance(inst, mybir.InstEventSemaphore):
                    cap = 2
                else:
                    cap = 1
                if si is not None and si.on_wait and len(si.on_wait) > cap:
                    waits = list(si.on_wait)
                    for w in waits[cap:]:
                        counter += 1
                        ev = mybir.InstEventSemaphore(name=f"I-wsplit-{counter}")
                        ev.engine = inst.engine
                        ev.sync_info = mybir.SyncInfo(on_wait=[w], on_update=[])
                        out.append(ev)
                    si.on_wait = waits[:cap]
                out.append(inst)
            if len(out) != len(insts):
                insts[:] = out
    return counter


P = 128
S = 1024
E = 768
H = 12
D = 64
KT = E // P        # 6 e-tiles
ST = S // P        # 8 s-tiles
NPAIR = H // 2     # 6 head pairs
QTILE = 512
NQ = S // QTILE    # 2 q-tiles
ESLICES = [(0, 512), (512, 256)]

f32 = mybir.dt.float32
f16 = mybir.dt.float16
bf16 = mybir.dt.bfloat16
EXP = mybir.ActivationFunctionType.Exp

_NC_CACHE = {}


def build(mm_dtype="f16", e_dtype="f16"):
    mdt = {"f16": f16, "bf16": bf16}[mm_dtype]
    edt = {"f16": f16, "bf16": bf16}[e_dtype]
    nc = bass.Bass()
    x_d = nc.declare_dram_parameter("x", [S, E], f32, isOutput=False)
    Wq_d = nc.declare_dram_parameter("Wq", [E, E], f32, isOutput=False)
    Wk_d = nc.declare_dram_parameter("Wk", [E, E], f32, isOutput=False)
    Wv_d = nc.declare_dram_parameter("Wv", [E, E], f32, isOutput=False)
    Wo_d = nc.declare_dram_parameter("Wo", [E, E], f32, isOutput=False)
    bq_d = nc.declare_dram_parameter("bq", [E], f32, isOutput=False)
    bk_d = nc.declare_dram_parameter("bk", [E], f32, isOutput=False)
    bv_d = nc.declare_dram_parameter("bv", [E], f32, isOutput=False)
    bo_d = nc.declare_dram_parameter("bo", [E], f32, isOutput=False)
    out_d = nc.declare_dram_parameter("out", [S, E], f32, isOutput=True)

    with ExitStack() as ctx:
        tc = ctx.enter_context(tile.TileContext(nc))
        singles = ctx.enter_context(tc.tile_pool(name="singles", bufs=1))
        xld = ctx.enter_context(tc.tile_pool(name="xld", bufs=8))
        x16p = ctx.enter_context(tc.tile_pool(name="x16p", bufs=4))
        wvst = ctx.enter_context(tc.tile_pool(name="wvst", bufs=6))
        wbig = ctx.enter_context(tc.tile_pool(name="wbig", bufs=1))
        qkp = ctx.enter_context(tc.tile_pool(name="qkp", bufs=2))
        ep = ctx.enter_context(tc.tile_pool(name="ep", bufs=2))
        np_pool = ctx.enter_context(tc.tile_pool(name="norm", bufs=2))
        outp = ctx.enter_context(tc.tile_pool(name="outp", bufs=2))
        bcast = ctx.enter_context(tc.tile_pool(name="bcast", bufs=1))
        # PSUM: S ([P,2,512]x2 = 4 banks) + mm ([P,512]x2) + att ([P,512]x2)
        psum = ctx.enter_context(tc.tile_pool(name="psum", bufs=2, space="PSUM"))

        # ---- persistent big buffers ----
        xT = singles.tile([P, KT, S], mdt)          # x^T  [e_in, s]
        V_ext = singles.tile([P, ST, H, P], edt)    # [s, ktile, head, ...]
        concatT = singles.tile([P, NPAIR, S], mdt)  # attn^T by pair
        v4 = V_ext[:].rearrange("p st (hh two) d -> p st hh two d", two=2)

        # ---- phase 0: DMAs ----
        # x tiles split across the sync and gpsimd DMA queues: one queue
        # issues+transfers at ~1.1us/tile plus ~2.5us completion-semaphore
        # lag, which paces the (now fast) fp16 transposes.
        x_sb = {}
        for st in range(ST):
            x_sb[st] = xld.tile([P, E], f32, tag="x", name="x_sb")
            nc.sync.dma_start(x_sb[st][:], x_d[st * P:(st + 1) * P, :])

        ident = singles.tile([P, P], f32)
        make_identity(nc, ident)

        def bcast_load(dst, src_ap):  # [E] -> [P, E] partition-step-0 DMA
            nc.gpsimd.dma_start(
                out=dst,
                in_=bass.AP(tensor=src_ap.tensor, offset=src_ap.offset,
                            ap=[[0, P]] + [list(a) for a in src_ap.ap]))

        # gpsimd queue: Wv staging (no ring WAR: bufs=6), bcasts, memsets
        Wv_sb = wbig.tile([P, KT, E], mdt, tag="wbig")
        Wv_re = Wv_d[:].rearrange("(ko p) m -> p ko m", p=P)
        wv_stg = []
        for j in range(KT):
            stg = wvst.tile([P, E], f32, tag="wvstage", name="wvstage")
            wv_stg.append(stg)
            nc.gpsimd.dma_start(stg[:], Wv_re[:, j, :])
        bv_bc = bcast.tile([P, E], f32, tag="bvbc")
        bcast_load(bv_bc[:], bv_d[:])
        for st in range(ST):
            nc.gpsimd.memset(v4[:, st, :, 0, D:P], 1.0)
            nc.gpsimd.memset(v4[:, st, :, 1, 0:D], 1.0)
        bo_bc = bcast.tile([P, E], f32, tag="bobc")
        bcast_load(bo_bc[:], bo_d[:])
        # Wv chunk casts on DVE.  NOT on ACT: any f16-writing ACT Copy
        # makes walrus pick a different activation table (load 1283 ->
        # 1539ns) that slows EVERY exp 1114 -> 1335ns (~+21us of ACT).
        # f32->f32 ACT copies (the norm sum-swaps) are fine.
        # x casts for the first tiles lead the DVE queue — the Wv casts
        # wait on gpsimd DMA sems (~11-15us) and would otherwise block
        # the transposes' cast chain on the in-order DVE queue.
        for j in range(KT):
            nc.vector.tensor_copy(Wv_sb[:, j, :], wv_stg[j][:])
        # small bias loads: 4B-element strided DMAs measured ~9.4us on
        # the DMA engines — parked on the gpsimd queue (idle after the
        # memsets; first consumer is proj_one's evac at ~50us).
        bq_sb = singles.tile([P, KT], f32)
        bk_sb = singles.tile([P, KT], f32)
        nc.gpsimd.dma_start(bq_sb[:], bq_d[:].rearrange("(o p) -> p o", p=P))
        nc.gpsimd.dma_start(bk_sb[:], bk_d[:].rearrange("(o p) -> p o", p=P))

        # ---- phase 1+2: transpose x and V-proj, interleaved per s-tile ----
        # All PSUM tiles are single-bank [P,512] under one tag so the same
        # 4-deep ring serves transposes, V-proj and phase 3's scores.
        def transpose_st(st):
            # PE transpose mode, fp32 direct from the DMA'd x tile.  An
            # fp16 variant (DVE pre-cast + 1-cycle/row streams) cut ~8us
            # of PE but made phase 1 x-semaphore-paced: DMA completion
            # jitter fed straight into PE idle + p-state resets, swinging
            # runs 217-244us.  The fp32 path hides the jitter (222+-1us).
            pt = psum.tile([P, 2, 512], f32, tag="S", name="pt")
            for e0, cnt, g in ((0, 4, 0), (4, 2, 1)):
                for j in range(cnt):
                    nc.tensor.transpose(
                        pt[:, g, j * P:(j + 1) * P],
                        x_sb[st][:, (e0 + j) * P:(e0 + j + 1) * P],
                        ident[:],
                    )
            nc.vector.tensor_copy(
                xT[:, 0:4, st * P:(st + 1) * P],
                pt[:, 0, :].rearrange("p (c s) -> p c s", c=4),
            )
            nc.vector.tensor_copy(
                xT[:, 4:6, st * P:(st + 1) * P],
                pt[:, 1, 0:256].rearrange("p (c s) -> p c s", c=2),
            )

        def vproj_st(st):
            # V-proj borrows the att/mm PSUM tags (idle until phase 3) so
            # its ring never couples to the transpose ring — transposes
            # and V-proj pipeline independently.
            pv0 = psum.tile([P, 512], f32, tag="att", name="pv0")
            pv1 = psum.tile([P, 512], f32, tag="mm", name="pv1")
            for k in range(KT):  # k-outer: xT stationary reused across nsi
                nc.tensor.matmul(
                    pv0[:], xT[:, k, st * P:(st + 1) * P],
                    Wv_sb[:, k, 0:512],
                    start=(k == 0), stop=(k == KT - 1),
                    skip_group_check=True)
                nc.tensor.matmul(
                    pv1[:, 0:256], xT[:, k, st * P:(st + 1) * P],
                    Wv_sb[:, k, 512:768],
                    start=(k == 0), stop=(k == KT - 1),
                    skip_group_check=True)
            # batched scatter: evens -> [V|ones] cols 0:64, odds -> 64:128
            for nsi, (noff, nsz) in enumerate(ESLICES):
                nh = nsz // P
                hh0 = 4 * nsi
                pv = pv0 if nsi == 0 else pv1
                pvr = pv[:, :nsz].rearrange(
                    "p (hh two d) -> p hh two d", two=2, d=D)
                bvr = bv_bc[:, noff:noff + nsz].rearrange(
                    "p (hh two d) -> p hh two d", two=2, d=D)
                nc.vector.tensor_add(
                    v4[:, st, hh0:hh0 + nh, 0, 0:D], pvr[:, :, 0, :],
                    bvr[:, :, 0, :])
                nc.vector.tensor_add(
                    v4[:, st, hh0:hh0 + nh, 1, D:P], pvr[:, :, 1, :],
                    bvr[:, :, 1, :])

        # s-tiles 0-3 transpose on PE (fp32, direct from the DMA'd x
        # tile — shortest startup chain); s-tiles 4-7 go through the XBAR
        # DMA transpose (DVE f32->f16 cast, then InstDmaTransposeAnt on
        # the scalar hwdge queue, ~14ns per 16x128 tile on the DMA
        # engines) — their timing is loose and this halves phase-1 PE
        # work.
        x16 = {}
        for st in range(4, ST):
            x16[st] = x16p.tile([P, E], mdt, tag="x16", name="x16")
            nc.vector.tensor_copy(x16[st][:], x_sb[st][:])
        for st in range(4, ST):
            nc.scalar.dma_start_transpose(
                xT[:, :, st * P:(st + 1) * P], x16[st][:])
        transpose_st(0)
        transpose_st(1)
        for st in range(ST):
            vproj_st(st)
            if st + 2 < 4:
                transpose_st(st + 2)

        # ---- phase 3: head pairs, software-pipelined ----
        # Wq/Wk live as persistent fp16 copies staged once through big
        # contiguous-chunk DMAs (the per-pair [P,KT,128] strided loads
        # cost 2.4-3.4us each on the DMA engines at 512B elements and
        # chained every pair's projection behind a DMA+cast).
        qt_t, kt_t = {}, {}
        Wq_re = Wq_d[:].rearrange("(ko p) m -> p ko m", p=P)
        Wk_re = Wk_d[:].rearrange("(ko p) m -> p ko m", p=P)
        Wq16 = singles.tile([P, KT, E], mdt)
        Wk16 = singles.tile([P, KT, E], mdt)

        def proj_fillers(m, which):
            """Yield one-PE-MM callables for QT_m/KT_m (the pq evacuation
            rides along with the last matmul of each q-half)."""
            w = Wq16 if which == "q" else Wk16
            bias = bq_sb if which == "q" else bk_sb
            t = qkp.tile([P, S], mdt, tag=which + "t", name=which + "t")
            (qt_t if which == "q" else kt_t)[m] = t
            for q2 in range(NQ):
                qsl = slice(q2 * QTILE, (q2 + 1) * QTILE)
                pq = psum.tile([P, 512], f32, tag="mm", name="pq")
                for k in range(KT):
                    def mm(k=k, pq=pq, qsl=qsl):
                        nc.tensor.matmul(pq[:], w[:, k, m * P:(m + 1) * P],
                                         xT[:, k, qsl],
                                         start=(k == 0), stop=(k == KT - 1),
                                         skip_group_check=True)
                        if k == KT - 1:
                            nc.vector.tensor_scalar_add(t[:, qsl], pq[:],
                                                        bias[:, m:m + 1])
                    yield mm

        def wload_big(dst16, src_re):
            # stages cycle through the same 6-deep ring as Wv: a chunk's
            # DMA only waits the cast 6 chunks back, keeping the DMA
            # queue ahead of the (busy) DVE cast stream.
            for j in range(KT):
                stg = wvst.tile([P, E], f32, tag="wvstage", name="wvstage")
                nc.sync.dma_start(stg[:], src_re[:, j, :])
                nc.vector.tensor_copy(dst16[:, j, :], stg[:])

        def outproj_fillers(sts):
            """Yield one-PE-MM callables for the output projection of the
            given s-tiles (bias add + store ride along with last matmuls)."""
            for st in sts:
                o_sb = outp.tile([P, E], f32, tag="o")
                for nsi, (noff, nsz) in enumerate(ESLICES):
                    po = psum.tile([P, 512], f32, tag="mm", name="po")
                    for k in range(KT):
                        def mm(k=k, po=po, st=st, nsi=nsi, noff=noff,
                               nsz=nsz, o_sb=o_sb):
                            nc.tensor.matmul(
                                po[:, :nsz],
                                concatT[:, k, st * P:(st + 1) * P],
                                Wo_sb[:, k, noff:noff + nsz],
                                start=(k == 0), stop=(k == KT - 1),
                                skip_group_check=True)
                            if k == KT - 1:
                                nc.vector.tensor_add(
                                    o_sb[:, noff:noff + nsz], po[:, :nsz],
                                    bo_bc[:, noff:noff + nsz])
                                if nsi == 1:
                                    nc.sync.dma_start(
                                        out_d[st * P:(st + 1) * P, :],
                                        o_sb[:])
                        yield mm

        wload_big(Wq16, Wq_re)
        wload_big(Wk16, Wk_re)
        for f in proj_fillers(0, "q"):
            f()
        for f in proj_fillers(0, "k"):
            f()
        # Wo loaded+cast here: overlaps the attention phase; the wbig slot
        # becomes free once the last V-proj matmul has read Wv.
        Wo_sb = wbig.tile([P, KT, E], mdt, tag="wbig")
        wload_big(Wo_sb, Wo_d[:].rearrange("(ko p) m -> p ko m", p=P))
        NOT_K = ~0x7EF311C2
        i32 = mybir.dt.int32
        AO = mybir.AluOpType
        units = [(m, q2) for m in range(NPAIR) for q2 in range(NQ)]
        e_t = {}

        def norm_unit(u, p_a, p_b):
            """Softmax normalization for unit u from its attnV PSUM tiles.
            The sums rows are copied into the attn rows' partitions FIRST
            (ACT Copy reads PSUM at a different partition base correctly,
            HW-verified; DVE cannot, and f32->f32 Copy keeps the exp
            activation table), so the Newton chain runs full-width and
            partition-aligned: 1/sums via a bit-trick seed (r0_bits =
            K - s_bits, rel err ~5%) + one Newton step r1 = r0*(2 - s*r0)
            (~2.6e-3, below the fp16 concatT quantization).  Signs:
            rr = (t-2)*r0 = -r1, applied as (-p)*(-r1)."""
            m, q2 = units[u]
            qsl = slice(q2 * QTILE, (q2 + 1) * QTILE)
            s_sw = np_pool.tile([P, 512], f32, tag="ssw")
            r0 = np_pool.tile([P, 512], f32, tag="r0")
            tt = np_pool.tile([P, 512], f32, tag="tt", bufs=1)
            rr = np_pool.tile([P, 512], f32, tag="rr")
            nc.scalar.copy(s_sw[0:D, :], p_a[D:P, :])
            nc.scalar.copy(s_sw[D:P, :], p_b[0:D, :])
            nc.vector.tensor_scalar(
                tt[:].bitcast(i32), s_sw[:].bitcast(i32),
                NOT_K, None, op0=AO.add)
            nc.vector.tensor_scalar(
                r0[:].bitcast(i32), tt[:].bitcast(i32),
                -1, None, op0=AO.bitwise_xor)
            nc.vector.tensor_mul(tt[:], s_sw[:], r0[:])
            nc.vector.scalar_tensor_tensor(
                rr[:], tt[:], 2.0, r0[:], op0=AO.subtract, op1=AO.mult)
            nc.vector.scalar_tensor_tensor(
                concatT[0:D, m, qsl], p_a[0:D, :], -1.0, rr[0:D, :],
                op0=AO.mult, op1=AO.mult)
            nc.vector.scalar_tensor_tensor(
                concatT[D:P, m, qsl], p_b[D:P, :], -1.0, rr[D:P, :],
                op0=AO.mult, op1=AO.mult)

        # Phase-3 unit loop (v8 structure — measured faster than both a
        # per-ktile interleave and a one-unit-deep software pipeline):
        # the next pair's projection leads each unit so PE has
        # independent work while ACT drains the previous unit's exps;
        # the q-half-0 output projection slots between the last unit's
        # scores and its attnV.
        for u in range(len(units)):
            m, q2 = units[u]
            qsl = slice(q2 * QTILE, (q2 + 1) * QTILE)
            if m + 1 < NPAIR:
                for f in proj_fillers(m + 1, "q" if q2 == 0 else "k"):
                    f()
            qt_m, kt_m = qt_t[m], kt_t[m]
            e_a = ep.tile([P, ST, QTILE], edt, tag="eA")
            e_b = ep.tile([P, ST, QTILE], edt, tag="eB")
            for c in range(ST // 2):
                s_a = psum.tile([P, 2, 512], f32, tag="S", name="s_a")
                s_b = psum.tile([P, 2, 512], f32, tag="S", name="s_b")
                for kk in range(2):
                    ktile = c * 2 + kk
                    ksl = slice(ktile * P, (ktile + 1) * P)
                    nc.tensor.matmul(s_a[:, kk, :], kt_m[0:D, ksl],
                                     qt_m[0:D, qsl], start=True, stop=True,
                                     skip_group_check=True)
                    nc.tensor.matmul(s_b[:, kk, :], kt_m[D:P, ksl],
                                     qt_m[D:P, qsl], start=True, stop=True,
                                     skip_group_check=True)
                nc.scalar.activation(e_a[:, c * 2:c * 2 + 2, :], s_a[:],
                                     EXP, scale=0.125)
                nc.scalar.activation(e_b[:, c * 2:c * 2 + 2, :], s_b[:],
                                     EXP, scale=0.125)
            if u == len(units) - 1:
                for f in outproj_fillers(range(4)):
                    f()
            p_a = psum.tile([P, 512], f32, tag="att", name="p_a")
            p_b = psum.tile([P, 512], f32, tag="att", name="p_b")
            for ktile in range(ST):
                nc.tensor.matmul(p_a[:], V_ext[:, ktile, 2 * m, :],
                                 e_a[:, ktile, :],
                                 start=(ktile == 0), stop=(ktile == ST - 1),
                                 skip_group_check=True)
            for ktile in range(ST):
                nc.tensor.matmul(p_b[:], V_ext[:, ktile, 2 * m + 1, :],
                                 e_b[:, ktile, :],
                                 start=(ktile == 0), stop=(ktile == ST - 1),
                                 skip_group_check=True)
            norm_unit(u, p_a, p_b)

        # ---- phase 4: output projection, remaining s-tiles ----
        for f in outproj_fillers(range(4, ST)):
            f()

    _split_excess_waits(nc)
    return nc


def run_spmd(inputs, Wq, bq, Wk, bk, Wv, bv, Wo, bo,
             mm_dtype="f16", e_dtype="f16", trace=False):
    key = (mm_dtype, e_dtype)
    if key not in _NC_CACHE:
        _NC_CACHE[key] = build(mm_dtype, e_dtype)
    nc = _NC_CACHE[key]
    x = np.asarray(inputs, dtype=np.float32)
    common = {
        "Wq": np.asarray(Wq, np.float32), "Wk": np.asarray(Wk, np.float32),
        "Wv": np.asarray(Wv, np.float32), "Wo": np.asarray(Wo, np.float32),
        "bq": np.asarray(bq, np.float32), "bk": np.asarray(bk, np.float32),
        "bv": np.asarray(bv, np.float32), "bo": np.asarray(bo, np.float32),
    }
    in_maps = [dict(common, x=np.ascontiguousarray(x[b])) for b in range(x.shape[0])]
    res = run_bass_kernel_spmd(nc, in_maps, core_ids=list(range(len(in_maps))),
                               trace=trace)
    out = np.stack([res.results[b]["out"] for b in range(len(in_maps))], axis=0)
    return out, res


def kernel(inputs, Wq, bq, Wk, bk, Wv, bv, Wo, bo):
    out, _ = run_spmd(inputs, Wq, bq, Wk, bk, Wv, bv, Wo, bo)
    return out


# revision 38
# speedup vs baseline: 1.0597x; 1.0597x over previous
"""Multi-head self-attention (B=8, S=1024, E=768, H=12, D=64) on 8 NeuronCores.

Sharding: data-parallel over batch — one batch element per core, weights
replicated, no collectives.  Measured: ~222us HW exec (v1 baseline
~233us), rel-RMS err ~1.6e-3 vs the fp32 reference.

Measured facts driving the design (NTFF profiles + probe kernels):
 - PE stream: fp16 moving operands cross the array at 1 cycle/row
   (~215ns for N=512 @2.4GHz); fp32 at 2 cycles/row.  All matmul
   operands are fp16.  ~40ns of exposed LDWEIGHTS per matmul (walrus
   can't dedup or hide them fully; --enable-ldw-opt rejects
   pre-legalized LDWEIGHTS).  The PE runs at 0.65/1.2GHz p-states until
   ~3us of gapless execution, only then 2.4GHz — so every PE bubble
   also slows the next ~3us of matmuls; the phase-3 emission order
   below exists to keep PE continuously fed.
 - PE busy is the floor engine (~184us); ACT exp is second: 12.6M exps
   at 1 col/cycle/partition @0.96GHz + ~50ns/instr = ~107us, plus the
   24 sum-copies.  Per (pair, q-half) unit, PE ~11.5us vs ACT ~10.3us —
   the target_regime "ridge".
 - XBAR DMA transposes (InstDmaTransposeAnt) for x^T were tried and
   REVERTED (v2-v4, v13-hybrid): each DMA stage pays a completion-
   semaphore lag (~2-6us), the limited DMA-semaphore pool serializes
   cross-queue, and x-DMA jitter feeds straight into PE idle+p-state
   resets (runs swung 217-292us).  PE transpose mode (~430ns/128x128
   fp32 tile, LDW direct from the DMA'd x tile) wins on the critical
   startup path and is jitter-tolerant (222+-1us typical).  An fp16 PE
   transpose variant (DVE pre-cast) cut ~8us PE but re-introduced the
   x-semaphore pacing and its variance.
 - ACT table trap: using Ln anywhere switches walrus to the combined
   natural_log_exp table (every exp 1116 -> 1281ns); any f16-writing
   ACT Copy switches tables too (exp -> 1335ns).  So ACT runs ONLY
   f32->f32 Copy and Exp; softmax normalization is Newton-on-DVE.
 - DVE ops must be partition-aligned across operands and output
   (verifier NCC_IBIR297; DVE PSUM reads at a different partition base
   silently corrupt).  ACT Copy DOES read PSUM cross-partition-base
   correctly (HW-verified), so the sums rows are copied into the attn
   rows' partitions by 2 ACT copies and the whole Newton chain runs
   full-width and aligned: 6 DVE ops/unit vs v1's 8 + 2 swap DMAs.
 - Per-pair [P,KT,128] weight loads are 512B-element strided DMAs
   (2.4-3.4us each on the DMA engines) — Wq/Wk instead live as
   persistent fp16 copies staged once through contiguous [P,768]
   chunks.  The bq/bk rearranges are 4B-element strided (~9.4us!) and
   are parked on the gpsimd queue.  Wv stages through the gpsimd DMA
   queue (bufs=6, no ring WAR); all weight staging shares that ring.

Per-core dataflow (layouts chosen so the only transpose is x -> xT):
  1. x s-tiles DMA in (one [128,768] DMA each, sync queue); xT = x^T
     via PE transpose (48 128x128 fp32 tiles), cast to fp16 on the
     PSUM->SBUF DVE evacuation.  Transposes interleave with V-proj per
     s-tile; V-proj borrows the att/mm PSUM tags (idle until phase 3)
     so the transpose and V-proj rings pipeline independently.
  2. V scattered into V_ext[s, ktile, head, 128] = [V_h+bv | ones]
     (even head) or [ones | V_h+bv] (odd head); ones-halves memset
     strided on gpsimd; adding bv here is exact (softmax rows sum to 1).
  3. QT/KT per head pair: lhsT=W tile, rhs=xT; bias via
     tensor_scalar_add on the PSUM evacuation (fp16 out).  The next
     pair's projection leads each (pair, q-half) unit so PE has
     independent work while ACT drains the previous unit's exps out of
     the S-ring.  (Both a per-ktile scores/attnV interleave and a
     one-unit-deep software pipeline measured SLOWER than this block
     order: 16-instr exps pay +160ns/instr ACT overhead, and fine
     PE<->ACT coupling converts ACT jitter into PE stalls.)
  4. Per (pair, q-half): scores^T[k,q] = KT.T @ QT (row halves at
     partition base 0/64), exp on ACT with the 1/sqrt(D)=1/8 scale
     folded in (scores ~ N(0,1), no max guard needed).
  5. attnV: one M=128 matmul per (head, ktile) -> rows [attn^T|sums]
     (even) / [sums|attn^T] (odd).  Normalization: 2 ACT copies swap
     the sums into the attn rows' partitions, then 1/sums via a
     bit-trick seed (r0_bits = K - s_bits) + one Newton step on DVE
     (~2.6e-3, below the fp16 concatT quantization).
  6. out = concatT.T @ Wo + bo (bo broadcast via partition-step-0 DMA).
     Output s-tiles 0-3 only need q-half 0 of concatT: their matmuls
     slot between the last unit's scores (which feed ACT) and its
     attnV (which waits on those exps), leaving s-tiles 4-7 plus one
     normalization chain for the tail.

Workaround baked in: this walrus build rejects instructions carrying
more than ~1-2 sync waits; _split_excess_waits and the patched
TileContext tail hoist surplus waits onto standalone EVSEM ops.
"""
import sys
sys.path.insert(0, "/opt/trn_rl_repo")
from contextlib import ExitStack

import numpy as np

import concourse.bass as bass
import concourse.tile as tile
from concourse import mybir
from concourse.bass_utils import run_bass_kernel_spmd
from concourse.masks import make_identity
from concourse.vector_clock import ScopedClock


def _split_drain_and_barrier(self, tick_clock, wait_clock):
    """TileContext tail with the final drain's waits split one-per-instruction."""
    drain_inst = self.nc.sync.drain()
    wait_clock.add_sem_waits(
        drain_inst.ins, ScopedClock({None: tick_clock.global_clock})
    )
    si = drain_inst.ins.sync_info
    waits = list(si.on_wait) if si is not None and si.on_wait else []
    if len(waits) > 1:
        si.on_wait = []
        by_num = {h.num: h for h in self.sems.allocated().values()}
        for w in waits:
            self.nc.sync.wait_ge(by_num[w.id], w.wait_value)
    self.nc.all_engine_barrier()
    popped = self.nc._tile_sem_poison_stack.pop()
    assert popped is self._sem_poison
    self.nc.clear_and_free_semaphores(list(self.sems.allocated().values()))
    self.nc.all_engine_barrier()


tile.TileContext._drain_and_barrier = _split_drain_and_barrier


def _split_excess_waits(nc):
    """Hoist excess per-instruction sync waits into standalone EVSEM waits
    (this walrus build rejects >1 wait per instruction, >2 per EVSEM)."""
    counter = 0
    for f in nc.m.functions:
        for bb in f.blocks:
            insts = bb.instructions
            out = []
            for inst in insts:
                si = inst.sync_info
                if "DmaTranspose" in type(inst).__name__:
                    cap = 0
                elif isinstance(inst, mybir.InstEventSemaphore):
                    cap = 2
                else:
                    cap = 1
                if si is not None and si.on_wait and len(si.on_wait) > cap:
                    waits = list(si.on_wait)
                    for w in waits[cap:]:
                        counter += 1
                        ev = mybir.InstEventSemaphore(name=f"I-wsplit-{counter}")
                        ev.engine = inst.engine
                        ev.sync_info = mybir.SyncInfo(on_wait=[w], on_update=[])
                        out.append(ev)
                    si.on_wait = waits[:cap]
                out.append(inst)
            if len(out) != len(insts):
                insts[:] = out
    return counter


P = 128
S = 1024
E = 768
H = 12
D = 64
KT = E // P        # 6 e-tiles
ST = S // P        # 8 s-tiles
NPAIR = H // 2     # 6 head pairs
QTILE = 512
NQ = S // QTILE    # 2 q-tiles
ESLICES = [(0, 512), (512, 256)]

f32 = mybir.dt.float32
f16 = mybir.dt.float16
bf16 = mybir.dt.bfloat16
EXP = mybir.ActivationFunctionType.Exp

_NC_CACHE = {}


def build(mm_dtype="f16", e_dtype="f16"):
    mdt = {"f16": f16, "bf16": bf16}[mm_dtype]
    edt = {"f16": f16, "bf16": bf16}[e_dtype]
    nc = bass.Bass()
    x_d = nc.declare_dram_parameter("x", [S, E], f32, isOutput=False)
    Wq_d = nc.declare_dram_parameter("Wq", [E, E], f32, isOutput=False)
    Wk_d = nc.declare_dram_parameter("Wk", [E, E], f32, isOutput=False)
    Wv_d = nc.declare_dram_parameter("Wv", [E, E], f32, isOutput=False)
    Wo_d = nc.declare_dram_parameter("Wo", [E, E], f32, isOutput=False)
    bq_d = nc.declare_dram_parameter("bq", [E], f32, isOutput=False)
    bk_d = nc.declare_dram_parameter("bk", [E], f32, isOutput=False)
    bv_d = nc.declare_dram_parameter("bv", [E], f32, isOutput=False)
    bo_d = nc.declare_dram_parameter("bo", [E], f32, isOutput=False)
    out_d = nc.declare_dram_parameter("out", [S, E], f32, isOutput=True)

    with ExitStack() as ctx:
        tc = ctx.enter_context(tile.TileContext(nc))
        singles = ctx.enter_context(tc.tile_pool(name="singles", bufs=1))
        xld = ctx.enter_context(tc.tile_pool(name="xld", bufs=8))
        wvst = ctx.enter_context(tc.tile_pool(name="wvst", bufs=6))
        wbig = ctx.enter_context(tc.tile_pool(name="wbig", bufs=1))
        qkp = ctx.enter_context(tc.tile_pool(name="qkp", bufs=2))
        ep = ctx.enter_context(tc.tile_pool(name="ep", bufs=2))
        np_pool = ctx.enter_context(tc.tile_pool(name="norm", bufs=2))
        outp = ctx.enter_context(tc.tile_pool(name="outp", bufs=2))
        bcast = ctx.enter_context(tc.tile_pool(name="bcast", bufs=1))
        # PSUM: S ([P,2,512]x2 = 4 banks) + mm ([P,512]x2) + att ([P,512]x2)
        psum = ctx.enter_context(tc.tile_pool(name="psum", bufs=2, space="PSUM"))

        # ---- persistent big buffers ----
        xT = singles.tile([P, KT, S], mdt)          # x^T  [e_in, s]
        V_ext = singles.tile([P, ST, H, P], edt)    # [s, ktile, head, ...]
        concatT = singles.tile([P, NPAIR, S], mdt)  # attn^T by pair
        v4 = V_ext[:].rearrange("p st (hh two) d -> p st hh two d", two=2)

        # ---- phase 0: DMAs ----
        # x tiles split across the sync and gpsimd DMA queues: one queue
        # issues+transfers at ~1.1us/tile plus ~2.5us completion-semaphore
        # lag, which paces the (now fast) fp16 transposes.
        x_sb = {}
        for st in range(ST):
            x_sb[st] = xld.tile([P, E], f32, tag="x", name="x_sb")
            nc.sync.dma_start(x_sb[st][:], x_d[st * P:(st + 1) * P, :])

        ident = singles.tile([P, P], f32)
        make_identity(nc, ident)

        def bcast_load(dst, src_ap):  # [E] -> [P, E] partition-step-0 DMA
            nc.gpsimd.dma_start(
                out=dst,
                in_=bass.AP(tensor=src_ap.tensor, offset=src_ap.offset,
                            ap=[[0, P]] + [list(a) for a in src_ap.ap]))

        # gpsimd queue: Wv staging (no ring WAR: bufs=6), bcasts, memsets
        Wv_sb = wbig.tile([P, KT, E], mdt, tag="wbig")
        Wv_re = Wv_d[:].rearrange("(ko p) m -> p ko m", p=P)
        wv_stg = []
        for j in range(KT):
            stg = wvst.tile([P, E], f32, tag="wvstage", name="wvstage")
            wv_stg.append(stg)
            nc.gpsimd.dma_start(stg[:], Wv_re[:, j, :])
        bv_bc = bcast.tile([P, E], f32, tag="bvbc")
        bcast_load(bv_bc[:], bv_d[:])
        for st in range(ST):
            nc.gpsimd.memset(v4[:, st, :, 0, D:P], 1.0)
            nc.gpsimd.memset(v4[:, st, :, 1, 0:D], 1.0)
        bo_bc = bcast.tile([P, E], f32, tag="bobc")
        bcast_load(bo_bc[:], bo_d[:])
        # Wv chunk casts on DVE.  NOT on ACT: any f16-writing ACT Copy
        # makes walrus pick a different activation table (load 1283 ->
        # 1539ns) that slows EVERY exp 1114 -> 1335ns (~+21us of ACT).
        # f32->f32 ACT copies (the norm sum-swaps) are fine.
        # x casts for the first tiles lead the DVE queue — the Wv casts
        # wait on gpsimd DMA sems (~11-15us) and would otherwise block
        # the transposes' cast chain on the in-order DVE queue.
        for j in range(KT):
            nc.vector.tensor_copy(Wv_sb[:, j, :], wv_stg[j][:])
        # small bias loads: 4B-element strided DMAs measured ~9.4us on
        # the DMA engines — parked on the gpsimd queue (idle after the
        # memsets; first consumer is proj_one's evac at ~50us).
        bq_sb = singles.tile([P, KT], f32)
        bk_sb = singles.tile([P, KT], f32)
        nc.gpsimd.dma_start(bq_sb[:], bq_d[:].rearrange("(o p) -> p o", p=P))
        nc.gpsimd.dma_start(bk_sb[:], bk_d[:].rearrange("(o p) -> p o", p=P))

        # ---- phase 1+2: transpose x and V-proj, interleaved per s-tile ----
        # All PSUM tiles are single-bank [P,512] under one tag so the same
        # 4-deep ring serves transposes, V-proj and phase 3's scores.
        def transpose_st(st):
            # PE transpose mode, fp32 direct from the DMA'd x tile.  An
            # fp16 variant (DVE pre-cast + 1-cycle/row streams) cut ~8us
            # of PE but made phase 1 x-semaphore-paced: DMA completion
            # jitter fed straight into PE idle + p-state resets, swinging
            # runs 217-244us.  The fp32 path hides the jitter (222+-1us).
            pt = psum.tile([P, 2, 512], f32, tag="S", name="pt")
            for e0, cnt, g in ((0, 4, 0), (4, 2, 1)):
                for j in range(cnt):
                    nc.tensor.transpose(
                        pt[:, g, j * P:(j + 1) * P],
                        x_sb[st][:, (e0 + j) * P:(e0 + j + 1) * P],
                        ident[:],
                    )
            nc.vector.tensor_copy(
                xT[:, 0:4, st * P:(st + 1) * P],
                pt[:, 0, :].rearrange("p (c s) -> p c s", c=4),
            )
            nc.vector.tensor_copy(
                xT[:, 4:6, st * P:(st + 1) * P],
                pt[:, 1, 0:256].rearrange("p (c s) -> p c s", c=2),
            )

        def vproj_st(st):
            # V-proj borrows the att/mm PSUM tags (idle until phase 3) so
            # its ring never couples to the transpose ring — transposes
            # and V-proj pipeline independently.
            pv0 = psum.tile([P, 512], f32, tag="att", name="pv0")
            pv1 = psum.tile([P, 512], f32, tag="mm", name="pv1")
            for k in range(KT):  # k-outer: xT stationary reused across nsi
                nc.tensor.matmul(
                    pv0[:], xT[:, k, st * P:(st + 1) * P],
                    Wv_sb[:, k, 0:512],
                    start=(k == 0), stop=(k == KT - 1),
                    skip_group_check=True)
                nc.tensor.matmul(
                    pv1[:, 0:256], xT[:, k, st * P:(st + 1) * P],
                    Wv_sb[:, k, 512:768],
                    start=(k == 0), stop=(k == KT - 1),
                    skip_group_check=True)
            # batched scatter: evens -> [V|ones] cols 0:64, odds -> 64:128
            for nsi, (noff, nsz) in enumerate(ESLICES):
                nh = nsz // P
                hh0 = 4 * nsi
                pv = pv0 if nsi == 0 else pv1
                pvr = pv[:, :nsz].rearrange(
                    "p (hh two d) -> p hh two d", two=2, d=D)
                bvr = bv_bc[:, noff:noff + nsz].rearrange(
                    "p (hh two d) -> p hh two d", two=2, d=D)
                nc.vector.tensor_add(
                    v4[:, st, hh0:hh0 + nh, 0, 0:D], pvr[:, :, 0, :],
                    bvr[:, :, 0, :])
                nc.vector.tensor_add(
                    v4[:, st, hh0:hh0 + nh, 1, D:P], pvr[:, :, 1, :],
                    bvr[:, :, 1, :])

        transpose_st(0)
        transpose_st(1)
        for st in range(ST):
            vproj_st(st)
            if st + 2 < ST:
                transpose_st(st + 2)

        # ---- phase 3: head pairs, software-pipelined ----
        # Wq/Wk live as persistent fp16 copies staged once through big
        # contiguous-chunk DMAs (the per-pair [P,KT,128] strided loads
        # cost 2.4-3.4us each on the DMA engines at 512B elements and
        # chained every pair's projection behind a DMA+cast).
        qt_t, kt_t = {}, {}
        Wq_re = Wq_d[:].rearrange("(ko p) m -> p ko m", p=P)
        Wk_re = Wk_d[:].rearrange("(ko p) m -> p ko m", p=P)
        Wq16 = singles.tile([P, KT, E], mdt)
        Wk16 = singles.tile([P, KT, E], mdt)

        def proj_fillers(m, which):
            """Yield one-PE-MM callables for QT_m/KT_m (the pq evacuation
            rides along with the last matmul of each q-half)."""
            w = Wq16 if which == "q" else Wk16
            bias = bq_sb if which == "q" else bk_sb
            t = qkp.tile([P, S], mdt, tag=which + "t", name=which + "t")
            (qt_t if which == "q" else kt_t)[m] = t
            for q2 in range(NQ):
                qsl = slice(q2 * QTILE, (q2 + 1) * QTILE)
                pq = psum.tile([P, 512], f32, tag="mm", name="pq")
                for k in range(KT):
                    def mm(k=k, pq=pq, qsl=qsl):
                        nc.tensor.matmul(pq[:], w[:, k, m * P:(m + 1) * P],
                                         xT[:, k, qsl],
                                         start=(k == 0), stop=(k == KT - 1),
                                         skip_group_check=True)
                        if k == KT - 1:
                            nc.vector.tensor_scalar_add(t[:, qsl], pq[:],
                                                        bias[:, m:m + 1])
                    yield mm

        def wload_big(dst16, src_re):
            # stages cycle through the same 6-deep ring as Wv: a chunk's
            # DMA only waits the cast 6 chunks back, keeping the DMA
            # queue ahead of the (busy) DVE cast stream.
            for j in range(KT):
                stg = wvst.tile([P, E], f32, tag="wvstage", name="wvstage")
                nc.sync.dma_start(stg[:], src_re[:, j, :])
                nc.vector.tensor_copy(dst16[:, j, :], stg[:])

        def outproj_fillers(sts):
            """Yield one-PE-MM callables for the output projection of the
            given s-tiles (bias add + store ride along with last matmuls)."""
            for st in sts:
                o_sb = outp.tile([P, E], f32, tag="o")
                for nsi, (noff, nsz) in enumerate(ESLICES):
                    po = psum.tile([P, 512], f32, tag="mm", name="po")
                    for k in range(KT):
                        def mm(k=k, po=po, st=st, nsi=nsi, noff=noff,
                               nsz=nsz, o_sb=o_sb):
                            nc.tensor.matmul(
                                po[:, :nsz],
                                concatT[:, k, st * P:(st + 1) * P],
                                Wo_sb[:, k, noff:noff + nsz],
                                start=(k == 0), stop=(k == KT - 1),
                                skip_group_check=True)
                            if k == KT - 1:
                                nc.vector.tensor_add(
                                    o_sb[:, noff:noff + nsz], po[:, :nsz],
                                    bo_bc[:, noff:noff + nsz])
                                if nsi == 1:
                                    nc.sync.dma_start(
                                        out_d[st * P:(st + 1) * P, :],
                                        o_sb[:])
                        yield mm

        wload_big(Wq16, Wq_re)
        wload_big(Wk16, Wk_re)
        for f in proj_fillers(0, "q"):
            f()
        for f in proj_fillers(0, "k"):
            f()
        # Wo loaded+cast here: overlaps the attention phase; the wbig slot
        # becomes free once the last V-proj matmul has read Wv.
        Wo_sb = wbig.tile([P, KT, E], mdt, tag="wbig")
        wload_big(Wo_sb, Wo_d[:].rearrange("(ko p) m -> p ko m", p=P))
        NOT_K = ~0x7EF311C2
        i32 = mybir.dt.int32
        AO = mybir.AluOpType
        units = [(m, q2) for m in range(NPAIR) for q2 in range(NQ)]
        e_t = {}

        def norm_unit(u, p_a, p_b):
            """Softmax normalization for unit u from its attnV PSUM tiles.
            The sums rows are copied into the attn rows' partitions FIRST
            (ACT Copy reads PSUM at a different partition base correctly,
            HW-verified; DVE cannot, and f32->f32 Copy keeps the exp
            activation table), so the Newton chain runs full-width and
            partition-aligned: 1/sums via a bit-trick seed (r0_bits =
            K - s_bits, rel err ~5%) + one Newton step r1 = r0*(2 - s*r0)
            (~2.6e-3, below the fp16 concatT quantization).  Signs:
            rr = (t-2)*r0 = -r1, applied as (-p)*(-r1)."""
            m, q2 = units[u]
            qsl = slice(q2 * QTILE, (q2 + 1) * QTILE)
            s_sw = np_pool.tile([P, 512], f32, tag="ssw")
            r0 = np_pool.tile([P, 512], f32, tag="r0")
            tt = np_pool.tile([P, 512], f32, tag="tt", bufs=1)
            rr = np_pool.tile([P, 512], f32, tag="rr")
            nc.scalar.copy(s_sw[0:D, :], p_a[D:P, :])
            nc.scalar.copy(s_sw[D:P, :], p_b[0:D, :])
            nc.vector.tensor_scalar(
                tt[:].bitcast(i32), s_sw[:].bitcast(i32),
                NOT_K, None, op0=AO.add)
            nc.vector.tensor_scalar(
                r0[:].bitcast(i32), tt[:].bitcast(i32),
                -1, None, op0=AO.bitwise_xor)
            nc.vector.tensor_mul(tt[:], s_sw[:], r0[:])
            nc.vector.scalar_tensor_tensor(
                rr[:], tt[:], 2.0, r0[:], op0=AO.subtract, op1=AO.mult)
            nc.vector.scalar_tensor_tensor(
                concatT[0:D, m, qsl], p_a[0:D, :], -1.0, rr[0:D, :],
                op0=AO.mult, op1=AO.mult)
            nc.vector.scalar_tensor_tensor(
                concatT[D:P, m, qsl], p_b[D:P, :], -1.0, rr[D:P, :],
                op0=AO.mult, op1=AO.mult)

        # Phase-3 unit loop (v8 structure — measured faster than both a
        # per-ktile interleave and a one-unit-deep software pipeline):
        # the next pair's projection leads each unit so PE has
        # independent work while ACT drains the previous unit's exps;
        # the q-half-0 output projection slots between the last unit's
        # scores and its attnV.
        for u in range(len(units)):
            m, q2 = units[u]
            qsl = slice(q2 * QTILE, (q2 + 1) * QTILE)
            if m + 1 < NPAIR:
                for f in proj_fillers(m + 1, "q" if q2 == 0 else "k"):
                    f()
            qt_m, kt_m = qt_t[m], kt_t[m]
            e_a = ep.tile([P, ST, QTILE], edt, tag="eA")
            e_b = ep.tile([P, ST, QTILE], edt, tag="eB")
            for c in range(ST // 2):
                s_a = psum.tile([P, 2, 512], f32, tag="S", name="s_a")
                s_b = psum.tile([P, 2, 512], f32, tag="S", name="s_b")
                for kk in range(2):
                    ktile = c * 2 + kk
                    ksl = slice(ktile * P, (ktile + 1) * P)
                    nc.tensor.matmul(s_a[:, kk, :], kt_m[0:D, ksl],
                                     qt_m[0:D, qsl], start=True, stop=True,
                                     skip_group_check=True)
                    nc.tensor.matmul(s_b[:, kk, :], kt_m[D:P, ksl],
                                     qt_m[D:P, qsl], start=True, stop=True,
                                     skip_group_check=True)
                nc.scalar.activation(e_a[:, c * 2:c * 2 + 2, :], s_a[:],
                                     EXP, scale=0.125)
                nc.scalar.activation(e_b[:, c * 2:c * 2 + 2, :], s_b[:],
                                     EXP, scale=0.125)
            if u == len(units) - 1:
                for f in outproj_fillers(range(4)):
                    f()
            p_a = psum.tile([P, 512], f32, tag="att", name="p_a")
            p_b = psum.tile([P, 512], f32, tag="att", name="p_b")
            for ktile in range(ST):
                nc.tensor.matmul(p_a[:], V_ext[:, ktile, 2 * m, :],
                                 e_a[:, ktile, :],
                                 start=(ktile == 0), stop=(ktile == ST - 1),
                                 skip_group_check=True)
            for ktile in range(ST):
                nc.tensor.matmul(p_b[:], V_ext[:, ktile, 2 * m + 1, :],
                                 e_b[:, ktile, :],
                                 start=(ktile == 0), stop=(ktile == ST - 1),
                                 skip_group_check=True)
            norm_unit(u, p_a, p_b)

        # ---- phase 4: output projection, remaining s-tiles ----
        for f in outproj_fillers(range(4, ST)):
            f()

    _split_excess_waits(nc)
    return nc


def run_spmd(inputs, Wq, bq, Wk, bk, Wv, bv, Wo, bo,
             mm_dtype="f16", e_dtype="f16", trace=False):
    key = (mm_dtype, e_dtype)
    if key not in _NC_CACHE:
        _NC_CACHE[key] = build(mm_dtype, e_dtype)
    nc = _NC_CACHE[key]
    x = np.asarray(inputs, dtype=np.float32)
    common = {
        "Wq": np.asarray(Wq, np.float32), "Wk": np.asarray(Wk, np.float32),
        "Wv": np.asarray(Wv, np.float32), "Wo": np.asarray(Wo, np.float32),
        "bq": np.asarray(bq, np.float32), "bk": np.asarray(bk, np.float32),
        "bv": np.asarray(bv, np.float32), "bo": np.asarray(bo, np.float32),
    }
    in_maps = [dict(common, x=np.ascontiguousarray(x[b])) for b in range(x.shape[0])]
    res = run_bass_kernel_spmd(nc, in_maps, core_ids=list(range(len(in_maps))),
                               trace=trace)
    out = np.stack([res.results[b]["out"] for b in range(len(in_maps))], axis=0)
    return out, res


def kernel(inputs, Wq, bq, Wk, bk, Wv, bv, Wo, bo):
    out, _ = run_spmd(inputs, Wq, bq, Wk, bk, Wv, bv, Wo, bo)
    return out
